# revision 7
# baseline (speedup 1.0000x reference)
"""CLAM (gated-attention MIL) Trainium2 kernel — self-contained.

Contract: kernel(**inputs) takes the FULL inputs from reference.setup_inputs()
(x [131072, 1024] f32, label scalar, and the model weights) and returns the full
reference output tuple (logits [1,2], y_proba [1,2], raw_attention [2, N],
total_loss scalar), computed with the heavy O(N) work distributed across 8
NeuronCores (bag dim N sharded: 16384 rows per core).

Per-core device kernel (Bass/Tile, fp32r matmuls = 1 cyc/column on the PE):
  hT = relu(WcT.T @ xT + bc)          [512, NL]  (x pre-transposed on host)
  gates = tanh/sigmoid(WvuT.T @ hT)   [512, NL]  (Wv and Wu fused)
  abT = a * b                         [256, NL]
  AT  = WaT.T @ abT + ba              [2, NL]    -> raw attention (streamed out)
  eT  = exp(AT)  (|A| < ~2 for this distribution -> no max subtraction),
        s_chunk = rowsum(eT)          (ACT accumulator)
  p   = sum_n e[c,n] * h[n,:]         (DVE fused multiply-reduce against a
                                       partition-broadcast of e via DRAM bounce)
Host merges the per-core/per-chunk softmax partials exactly, computes bag
logits / y_proba, selects the top/bottom-50 instances from the returned raw
attention, recomputes those 100 h rows in numpy, and evaluates the
SmoothTop1SVM instance loss.
"""
import sys

for _p in ("/opt/trn_rl_repo", "/root/.axon_site/_ro/trn_rl_repo"):
    if _p not in sys.path:
        sys.path.append(_p)

import numpy as np

import concourse.bass as bass
from concourse import bacc
import concourse.mybir as mybir
import concourse.tile as tile
from concourse.bass_utils import run_bass_kernel_spmd

F32 = mybir.dt.float32
F32R = mybir.dt.float32r
BF16 = mybir.dt.bfloat16
AF = mybir.ActivationFunctionType
ALU = mybir.AluOpType

NB = 512          # rows per chunk
N_CORES = 8
N_INST = 50
TAU = 1.0
ALPHA = 1.0


def build_nc(NL, repeat=1):
    """Per-core Bass module for NL rows of x (NL % NB == 0).

    repeat>1 re-runs the whole chunk loop inside one NEFF (used only by
    test.py to measure per-iteration device time by slope)."""
    NCH = NL // NB
    nc = bacc.Bacc(trn_type="TRN2")

    xt = nc.dram_tensor("xt", [1024, NL], F32R, kind="ExternalInput")
    wc = nc.dram_tensor("wc", [128, 8 * 4 * 128], F32R, kind="ExternalInput")
    wvu = nc.dram_tensor("wvu", [128, 4 * 4 * 128], F32R, kind="ExternalInput")
    wa = nc.dram_tensor("wa", [128, 2 * 2], F32R, kind="ExternalInput")
    bcb = nc.dram_tensor("bcb", [128, 4], F32, kind="ExternalInput")
    bvub = nc.dram_tensor("bvub", [128, 4], F32, kind="ExternalInput")
    bab = nc.dram_tensor("bab", [2, 1], F32, kind="ExternalInput")

    a_out = nc.dram_tensor("a_out", [2, NL], F32, kind="ExternalOutput")
    p_out = nc.dram_tensor("p_out", [128, 8], F32, kind="ExternalOutput")
    s_out = nc.dram_tensor("s_out", [2, NCH], F32, kind="ExternalOutput")

    xt_r = xt.rearrange("(et p) n -> p et n", p=128)

    with tile.TileContext(nc) as tc:
        with (
            tc.tile_pool(name="singles", bufs=1) as singles,
            tc.tile_pool(name="xp", bufs=3) as xp,
            tc.tile_pool(name="hp", bufs=3) as hp,
            tc.tile_pool(name="gp", bufs=2) as gp,
            tc.tile_pool(name="abp", bufs=2) as abp,
            tc.tile_pool(name="etp", bufs=3) as etp,
            tc.tile_pool(name="ebcp", bufs=3) as ebcp,
            tc.tile_pool(name="junkp", bufs=2) as junkp,
            tc.tile_pool(name="accp", bufs=3) as accp,
            tc.tile_pool(name="php", bufs=3, space="PSUM") as php,
            tc.tile_pool(name="pabp", bufs=3, space="PSUM") as pabp,
            tc.tile_pool(name="patp", bufs=2, space="PSUM") as patp,
            tc.tile_pool(name="edp", bufs=3, space="DRAM") as edp,
        ):
            wc_sb = singles.tile([128, 8 * 4 * 128], F32R)
            for et in range(8):
                nc.sync.dma_start(out=wc_sb[:, et * 512:(et + 1) * 512],
                                  in_=wc[:, et * 512:(et + 1) * 512])
            wc4 = wc_sb.rearrange("p (et lt q) -> p et lt q", et=8, lt=4)
            wvu_sb = singles.tile([128, 4 * 4 * 128], F32R)
            nc.sync.dma_start(out=wvu_sb, in_=wvu[:, :])
            wvu4 = wvu_sb.rearrange("p (lt dt q) -> p lt dt q", lt=4, dt=4)
            wa_sb = singles.tile([128, 2 * 2], F32R)
            nc.sync.dma_start(out=wa_sb, in_=wa[:, :])
            wa4 = wa_sb.rearrange("p (dt c) -> p dt c", dt=2)
            bcb_sb = singles.tile([128, 4], F32)
            nc.sync.dma_start(out=bcb_sb, in_=bcb[:, :])
            bvub_sb = singles.tile([128, 4], F32)
            nc.sync.dma_start(out=bvub_sb, in_=bvub[:, :])
            bab_sb = singles.tile([2, 1], F32)
            nc.sync.dma_start(out=bab_sb, in_=bab[:, :])
            s_sb = singles.tile([2, NCH], F32)
            p_run = singles.tile([128, 8], F32)
            nc.vector.memset(p_run, 0.0)

            def emit_head(j):
                """xt load + h matmuls + relu for chunk j; returns (j, hT)."""
                xt_sb = xp.tile([128, 8, NB], F32R, tag="xt", name=f"xt{j}")
                if j == 0:
                    for et in range(8):
                        nc.sync.dma_start(out=xt_sb[:, et, :],
                                          in_=xt_r[:, et, j * NB:(j + 1) * NB])
                else:
                    nc.sync.dma_start(out=xt_sb, in_=xt_r[:, :, j * NB:(j + 1) * NB])
                hT = hp.tile([128, 4, NB], F32R, tag="hT", name=f"hT{j}")
                for lt in range(4):
                    ph = php.tile([128, NB], F32, tag="ph", name=f"ph{j}_{lt}")
                    for et in range(8):
                        nc.tensor.matmul(ph, wc4[:, et, lt, :], xt_sb[:, et, :],
                                         start=(et == 0), stop=(et == 7))
                    nc.scalar.activation(out=hT[:, lt, :], in_=ph, func=AF.Relu,
                                         bias=bcb_sb[:, lt:lt + 1], scale=1.0)
                return j, hT

            def emit_gates(state):
                """a/b matmuls + gate activations + ab product for chunk j."""
                j, hT = state
                gates = gp.tile([128, 4, NB], F32, tag="gates", name=f"g{j}")
                for dt in range(4):
                    pab = pabp.tile([128, NB], F32, tag="pab", name=f"pab{j}_{dt}")
                    for lt in range(4):
                        nc.tensor.matmul(pab, wvu4[:, lt, dt, :], hT[:, lt, :],
                                         start=(lt == 0), stop=(lt == 3))
                    nc.scalar.activation(out=gates[:, dt, :], in_=pab,
                                         func=(AF.Tanh if dt < 2 else AF.Sigmoid),
                                         bias=bvub_sb[:, dt:dt + 1], scale=1.0)
                abT = abp.tile([128, 2, NB], F32R, tag="abT", name=f"ab{j}")
                for dt2 in range(2):
                    nc.vector.tensor_mul(abT[:, dt2, :], gates[:, dt2, :], gates[:, dt2 + 2, :])
                return j, hT, abT

            def emit_tail(state):
                """attention scores, exp/stats and the p reduction for chunk j.

                Emitted one chunk late so the PE's in-order queue has the next
                chunk's h-matmuls before MM4's wait on the DVE ab product."""
                j, hT, abT = state
                pat = patp.tile([2, NB], F32, tag="pat", name=f"pat{j}")
                for dt2 in range(2):
                    nc.tensor.matmul(pat, wa4[:, dt2, :], abT[:, dt2, :],
                                     start=(dt2 == 0), stop=(dt2 == 1))
                at_sb = etp.tile([2, NB], F32, tag="at", name=f"at{j}")
                nc.vector.tensor_scalar_add(at_sb, pat, bab_sb[:, :])
                nc.sync.dma_start(out=a_out[:, j * NB:(j + 1) * NB], in_=at_sb)
                eT = etp.tile([2, NB], F32, tag="eT", name=f"eT{j}")
                nc.scalar.activation(out=eT, in_=pat, func=AF.Exp,
                                     bias=bab_sb[:, :], scale=1.0,
                                     accum_out=s_sb[:, j:j + 1])
                # p += e.T-weighted row sums of h, via DVE fused multiply-reduce.
                # e must be broadcast across partitions: bounce through DRAM
                # (DRAM APs permit a zero partition step; SBUF APs do not).
                e_dram = edp.tile([2, NB], F32, tag="edram", name=f"ed{j}")
                nc.sync.dma_start(out=e_dram, in_=eT)
                e_bc = ebcp.tile([128, 2, NB], F32, tag="ebc", name=f"ebc{j}")
                for c in range(2):
                    row = e_dram[c:c + 1, :]
                    bcast = bass.AP(tensor=row.tensor, offset=row.offset,
                                    ap=[[0, 128]] + list(row.ap[1:]))
                    nc.sync.dma_start(out=e_bc[:, c, :], in_=bcast)
                acc_c = accp.tile([128, 8], F32, tag="acc", name=f"acc{j}")
                for c in range(2):
                    for lt in range(4):
                        junk = junkp.tile([128, NB], F32, tag="junk", name=f"jk{j}_{c}_{lt}")
                        nc.vector.scalar_tensor_tensor(
                            out=junk, in0=hT.bitcast(F32)[:, lt, :], scalar=1.0,
                            in1=e_bc[:, c, :], op0=ALU.mult, op1=ALU.mult,
                            accum_out=acc_c[:, c * 4 + lt:c * 4 + lt + 1])
                nc.vector.tensor_add(p_run, p_run, acc_c)

            pending = None
            for _rep in range(repeat):
              for j in range(NCH):
                head = emit_head(j)
                if pending is not None:
                    emit_tail(pending)
                    pending = None
                pending = emit_gates(head)
              emit_tail(pending)
              pending = None

            nc.sync.dma_start(out=p_out[:, :], in_=p_run)
            nc.sync.dma_start(out=s_out[:, :], in_=s_sb)

    nc.compile()
    return nc


def prep_weights(Wc, bc, Wv, bv, Wu, bu, Wa, ba):
    f32 = np.float32
    WcT = np.ascontiguousarray(Wc.T, f32)                     # [1024, 512]
    wc_host = np.ascontiguousarray(
        WcT.reshape(8, 128, 4, 128).transpose(1, 0, 2, 3).reshape(128, -1))
    WvuT = np.concatenate([Wv.T, Wu.T], axis=1).astype(f32)   # [512, 512]
    wvu_host = np.ascontiguousarray(
        WvuT.reshape(4, 128, 4, 128).transpose(1, 0, 2, 3).reshape(128, -1))
    WaT = np.ascontiguousarray(Wa.T, f32)                     # [256, 2]
    wa_host = np.ascontiguousarray(
        WaT.reshape(2, 128, 2).transpose(1, 0, 2).reshape(128, 4))
    bcb = np.ascontiguousarray(bc.reshape(4, 128).T)
    bvub = np.ascontiguousarray(np.concatenate([bv, bu]).astype(f32).reshape(4, 128).T)
    bab = np.ascontiguousarray(ba.reshape(2, 1), f32)
    return dict(wc=wc_host, wvu=wvu_host, wa=wa_host, bcb=bcb, bvub=bvub, bab=bab)


_NC_CACHE = {}


def get_nc(NL):
    if NL not in _NC_CACHE:
        _NC_CACHE[NL] = build_nc(NL)
    return _NC_CACHE[NL]


def run_cores(x, weights, n_cores=N_CORES):
    """x: [N, 1024] f32. Returns A [2, N], p [2, 512] (f64), Z [2] (f64)."""
    N = x.shape[0]
    NL = N // n_cores
    nc = get_nc(NL)
    wmap = prep_weights(**weights)
    xT = np.ascontiguousarray(x.T)
    in_maps = []
    for c in range(n_cores):
        m = dict(wmap)
        m["xt"] = np.ascontiguousarray(xT[:, c * NL:(c + 1) * NL])
        in_maps.append(m)
    res = run_bass_kernel_spmd(nc, in_maps, core_ids=list(range(n_cores)))
    A = np.concatenate([res.results[c]["a_out"] for c in range(n_cores)], axis=1)
    p_raw = np.sum([res.results[c]["p_out"].astype(np.float64) for c in range(n_cores)], axis=0)
    p = p_raw.reshape(128, 2, 4).transpose(1, 2, 0).reshape(2, 512)
    Z = np.sum([res.results[c]["s_out"].astype(np.float64).sum(axis=1) for c in range(n_cores)], axis=0)
    return A, p, Z


def host_finalize(A, p, Z, x, Wc, bc, Winst, binst, Wbag, bbag, label):
    bag = (p / Z[:, None]).astype(np.float32)                         # [2, 512]
    logits = (np.sum(bag * Wbag, axis=1) + bbag)[None, :].astype(np.float32)
    ex = np.exp((logits - logits.max()).astype(np.float32))
    y_proba = (ex / ex.sum()).astype(np.float32)

    attn_c = A[label]
    top_p = np.argsort(-attn_c, kind="stable")[:N_INST]
    top_n = np.argsort(attn_c, kind="stable")[:N_INST]
    idx = np.concatenate([top_p, top_n])
    h_rows = np.maximum(x[idx] @ Wc.T + bc, 0.0).astype(np.float32)
    logits_i = (h_rows @ Winst[label].T + binst[label]).astype(np.float32)
    targets = np.concatenate([np.ones(N_INST, np.int64), np.zeros(N_INST, np.int64)])
    onehot = np.eye(2, dtype=np.float32)[targets]
    z = (logits_i + ALPHA * (1.0 - onehot)) / TAU
    zm = z.max(axis=1, keepdims=True)
    lse = TAU * (np.log(np.exp(z - zm).sum(axis=1)) + zm[:, 0])
    s_y = logits_i[np.arange(2 * N_INST), targets]
    total_loss = np.float32((lse - s_y).mean())
    return logits, y_proba, total_loss


def _host_softmax_stats(A, x, Wc, bc):
    """Fallback: exact bag stats from A on the host (used only if the device's
    unshifted exp over/underflowed, i.e. |A| was far outside the expected
    O(1) range for this model). Recomputes h in row blocks."""
    m = A.max(axis=1, keepdims=True)
    e = np.exp((A - m).astype(np.float64))
    Z = e.sum(axis=1)
    p = np.zeros((2, Wc.shape[0]), np.float64)
    B = 8192
    for i in range(0, x.shape[0], B):
        h = np.maximum(x[i:i + B] @ Wc.T + bc, 0.0).astype(np.float32)
        p += e[:, i:i + B] @ h.astype(np.float64)
    return p, Z


def kernel(**inputs):
    x = np.ascontiguousarray(np.asarray(inputs["x"]), dtype=np.float32)
    label = int(inputs["label"])
    weights = {k: np.asarray(inputs[k], np.float32)
               for k in ["Wc", "bc", "Wv", "bv", "Wu", "bu", "Wa", "ba"]}
    A, p, Z = run_cores(x, weights)
    if not (np.all(np.isfinite(p)) and np.all(np.isfinite(Z)) and Z.min() > 0
            and np.abs(A).max() < 60.0):
        p, Z = _host_softmax_stats(A, x, weights["Wc"], weights["bc"])
    logits, y_proba, total_loss = host_finalize(
        A, p, Z, x, weights["Wc"], weights["bc"],
        np.asarray(inputs["Winst"], np.float32), np.asarray(inputs["binst"], np.float32),
        np.asarray(inputs["Wbag"], np.float32), np.asarray(inputs["bbag"], np.float32),
        label)
    return logits, y_proba, A, total_loss


# revision 9
# speedup vs baseline: 2.2013x; 2.2013x over previous
"""CLAM (gated-attention MIL) Trainium2 kernel — self-contained.

Contract: kernel(**inputs) takes the FULL inputs from reference.setup_inputs()
(x [131072, 1024] f32, label scalar, and the model weights) and returns the full
reference output tuple (logits [1,2], y_proba [1,2], raw_attention [2, N],
total_loss scalar), computed with the heavy O(N) work distributed across 8
NeuronCores (bag dim N sharded: 16384 rows per core).

Per-core device kernel (Bass/Tile, fp32r matmuls = 1 cyc/column on the PE):
  hT = relu(WcT.T @ xT + bc)          [512, NL]  (x pre-transposed on host)
  gates = tanh/sigmoid(WvuT.T @ hT)   [512, NL]  (Wv and Wu fused)
  abT = a * b                         [256, NL]
  AT  = WaT.T @ abT + ba              [2, NL]    -> raw attention (streamed out)
  eT  = exp(AT)  (|A| < ~2 for this distribution -> no max subtraction),
        s_chunk = rowsum(eT)          (ACT accumulator)
  p   = sum_n e[c,n] * h[n,:]         (DVE fused multiply-reduce against a
                                       partition-broadcast of e via DRAM bounce)
Host merges the per-core/per-chunk softmax partials exactly, computes bag
logits / y_proba, selects the top/bottom-50 instances from the returned raw
attention, recomputes those 100 h rows in numpy, and evaluates the
SmoothTop1SVM instance loss.
"""
import sys

for _p in ("/opt/trn_rl_repo", "/root/.axon_site/_ro/trn_rl_repo"):
    if _p not in sys.path:
        sys.path.append(_p)

import numpy as np

import concourse.bass as bass
from concourse import bacc
import concourse.mybir as mybir
import concourse.tile as tile
from concourse.bass_utils import run_bass_kernel_spmd

F32 = mybir.dt.float32
F32R = mybir.dt.float32r
BF16 = mybir.dt.bfloat16
AF = mybir.ActivationFunctionType
ALU = mybir.AluOpType

NB = 512          # rows per chunk
N_CORES = 8
N_INST = 50
TAU = 1.0
ALPHA = 1.0


def build_nc(NL, repeat=1):
    """Per-core Bass module for NL rows of x (NL % NB == 0).

    repeat>1 re-runs the whole chunk loop inside one NEFF (used only by
    test.py to measure per-iteration device time by slope)."""
    NCH = NL // NB
    nc = bacc.Bacc(trn_type="TRN2")

    xt = nc.dram_tensor("xt", [1024, NL], F32R, kind="ExternalInput")
    wc = nc.dram_tensor("wc", [128, 8 * 4 * 128], F32R, kind="ExternalInput")
    wvu = nc.dram_tensor("wvu", [128, 4 * 4 * 128], F32R, kind="ExternalInput")
    wa = nc.dram_tensor("wa", [128, 2 * 2], F32R, kind="ExternalInput")
    bcb = nc.dram_tensor("bcb", [128, 4], F32, kind="ExternalInput")
    bvub = nc.dram_tensor("bvub", [128, 4], F32, kind="ExternalInput")
    bab = nc.dram_tensor("bab", [2, 1], F32, kind="ExternalInput")

    a_out = nc.dram_tensor("a_out", [2, NL], F32, kind="ExternalOutput")
    p_out = nc.dram_tensor("p_out", [128, 8 * (NL // NB)], F32, kind="ExternalOutput")
    s_out = nc.dram_tensor("s_out", [2, NCH], F32, kind="ExternalOutput")

    xt_r = xt.rearrange("(et p) n -> p et n", p=128)

    with tile.TileContext(nc) as tc:
        with (
            tc.tile_pool(name="singles", bufs=1) as singles,
            tc.tile_pool(name="xp", bufs=3) as xp,
            tc.tile_pool(name="hp", bufs=3) as hp,
            tc.tile_pool(name="gp", bufs=2) as gp,
            tc.tile_pool(name="abp", bufs=2) as abp,
            tc.tile_pool(name="etp", bufs=3) as etp,
            tc.tile_pool(name="ebcp", bufs=3) as ebcp,
            tc.tile_pool(name="junkp", bufs=2) as junkp,
            tc.tile_pool(name="accp", bufs=3) as accp,
            tc.tile_pool(name="php", bufs=3, space="PSUM") as php,
            tc.tile_pool(name="pabp", bufs=3, space="PSUM") as pabp,
            tc.tile_pool(name="patp", bufs=2, space="PSUM") as patp,
            tc.tile_pool(name="edp", bufs=3, space="DRAM") as edp,
        ):
            wc_sb = singles.tile([128, 8 * 4 * 128], F32R)
            for et in range(8):
                nc.sync.dma_start(out=wc_sb[:, et * 512:(et + 1) * 512],
                                  in_=wc[:, et * 512:(et + 1) * 512])
            wc4 = wc_sb.rearrange("p (et lt q) -> p et lt q", et=8, lt=4)
            wvu_sb = singles.tile([128, 4 * 4 * 128], F32R)
            nc.sync.dma_start(out=wvu_sb, in_=wvu[:, :])
            wvu4 = wvu_sb.rearrange("p (lt dt q) -> p lt dt q", lt=4, dt=4)
            wa_sb = singles.tile([128, 2 * 2], F32R)
            nc.sync.dma_start(out=wa_sb, in_=wa[:, :])
            wa4 = wa_sb.rearrange("p (dt c) -> p dt c", dt=2)
            bcb_sb = singles.tile([128, 4], F32)
            nc.sync.dma_start(out=bcb_sb, in_=bcb[:, :])
            bvub_sb = singles.tile([128, 4], F32)
            nc.sync.dma_start(out=bvub_sb, in_=bvub[:, :])
            bab_sb = singles.tile([2, 1], F32)
            nc.sync.dma_start(out=bab_sb, in_=bab[:, :])
            s_sb = singles.tile([2, NCH], F32)
            acc_all = singles.tile([128, NCH, 8], F32)
            a_all = singles.tile([2, NL], F32)

            def emit_head(j):
                """xt load + h matmuls + relu for chunk j; returns (j, hT)."""
                xt_sb = xp.tile([128, 8, NB], F32R, tag="xt", name=f"xt{j}")
                if j == 0:
                    for et in range(8):
                        nc.sync.dma_start(out=xt_sb[:, et, :],
                                          in_=xt_r[:, et, j * NB:(j + 1) * NB])
                else:
                    nc.sync.dma_start(out=xt_sb, in_=xt_r[:, :, j * NB:(j + 1) * NB])
                hT = hp.tile([128, 4, NB], F32R, tag="hT", name=f"hT{j}")
                for lt in range(4):
                    ph = php.tile([128, NB], F32, tag="ph", name=f"ph{j}_{lt}")
                    for et in range(8):
                        nc.tensor.matmul(ph, wc4[:, et, lt, :], xt_sb[:, et, :],
                                         start=(et == 0), stop=(et == 7))
                    nc.scalar.activation(out=hT[:, lt, :], in_=ph, func=AF.Relu,
                                         bias=bcb_sb[:, lt:lt + 1], scale=1.0)
                return j, hT

            def emit_gates(state):
                """a/b matmuls + gate activations + ab product for chunk j."""
                j, hT = state
                gates = gp.tile([128, 4, NB], F32, tag="gates", name=f"g{j}")
                for dt in range(4):
                    pab = pabp.tile([128, NB], F32, tag="pab", name=f"pab{j}_{dt}")
                    for lt in range(4):
                        nc.tensor.matmul(pab, wvu4[:, lt, dt, :], hT[:, lt, :],
                                         start=(lt == 0), stop=(lt == 3))
                    nc.scalar.activation(out=gates[:, dt, :], in_=pab,
                                         func=(AF.Tanh if dt < 2 else AF.Sigmoid),
                                         bias=bvub_sb[:, dt:dt + 1], scale=1.0)
                abT = abp.tile([128, 2, NB], F32R, tag="abT", name=f"ab{j}")
                for dt2 in range(2):
                    nc.vector.tensor_mul(abT[:, dt2, :], gates[:, dt2, :], gates[:, dt2 + 2, :])
                return j, hT, abT

            def emit_tail(state):
                """attention scores, exp/stats and the p reduction for chunk j.

                Emitted one chunk late so the PE's in-order queue has the next
                chunk's h-matmuls before MM4's wait on the DVE ab product."""
                j, hT, abT = state
                pat = patp.tile([2, NB], F32, tag="pat", name=f"pat{j}")
                for dt2 in range(2):
                    nc.tensor.matmul(pat, wa4[:, dt2, :], abT[:, dt2, :],
                                     start=(dt2 == 0), stop=(dt2 == 1))
                nc.vector.tensor_scalar_add(a_all[:, j * NB:(j + 1) * NB], pat, bab_sb[:, :])
                eT = etp.tile([2, NB], F32, tag="eT", name=f"eT{j}")
                nc.scalar.activation(out=eT, in_=pat, func=AF.Exp,
                                     bias=bab_sb[:, :], scale=1.0,
                                     accum_out=s_sb[:, j:j + 1])
                # p += e.T-weighted row sums of h, via DVE fused multiply-reduce.
                # e must be broadcast across partitions: bounce through DRAM
                # (DRAM APs permit a zero partition step; SBUF APs do not).
                e_dram = edp.tile([2, NB], F32, tag="edram", name=f"ed{j}")
                nc.sync.dma_start(out=e_dram, in_=eT)
                e_bc = ebcp.tile([128, 2, NB], F32, tag="ebc", name=f"ebc{j}")
                for c in range(2):
                    row = e_dram[c:c + 1, :]
                    bcast = bass.AP(tensor=row.tensor, offset=row.offset,
                                    ap=[[0, 128]] + list(row.ap[1:]))
                    nc.sync.dma_start(out=e_bc[:, c, :], in_=bcast)
                for c in range(2):
                    for lt in range(4):
                        junk = junkp.tile([128, NB], F32, tag="junk", name=f"jk{j}_{c}_{lt}")
                        nc.vector.scalar_tensor_tensor(
                            out=junk, in0=hT.bitcast(F32)[:, lt, :], scalar=1.0,
                            in1=e_bc[:, c, :], op0=ALU.mult, op1=ALU.mult,
                            accum_out=acc_all[:, j, c * 4 + lt:c * 4 + lt + 1])

            pending = None
            for _rep in range(repeat):
              for j in range(NCH):
                head = emit_head(j)
                if pending is not None:
                    emit_tail(pending)
                    pending = None
                if j > 0 and j % 8 == 0:
                    g0 = (j - 8) * NB
                    nc.sync.dma_start(out=a_out[:, g0:j * NB], in_=a_all[:, g0:j * NB])
                pending = emit_gates(head)
              emit_tail(pending)
              pending = None
              g0 = (NCH - 8) * NB
              nc.sync.dma_start(out=a_out[:, g0:NCH * NB], in_=a_all[:, g0:NCH * NB])

            nc.sync.dma_start(out=p_out[:, :], in_=acc_all.rearrange("p a b -> p (a b)"))
            nc.sync.dma_start(out=s_out[:, :], in_=s_sb)

    nc.compile()
    return nc


def prep_weights(Wc, bc, Wv, bv, Wu, bu, Wa, ba):
    f32 = np.float32
    WcT = np.ascontiguousarray(Wc.T, f32)                     # [1024, 512]
    wc_host = np.ascontiguousarray(
        WcT.reshape(8, 128, 4, 128).transpose(1, 0, 2, 3).reshape(128, -1))
    WvuT = np.concatenate([Wv.T, Wu.T], axis=1).astype(f32)   # [512, 512]
    wvu_host = np.ascontiguousarray(
        WvuT.reshape(4, 128, 4, 128).transpose(1, 0, 2, 3).reshape(128, -1))
    WaT = np.ascontiguousarray(Wa.T, f32)                     # [256, 2]
    wa_host = np.ascontiguousarray(
        WaT.reshape(2, 128, 2).transpose(1, 0, 2).reshape(128, 4))
    bcb = np.ascontiguousarray(bc.reshape(4, 128).T)
    bvub = np.ascontiguousarray(np.concatenate([bv, bu]).astype(f32).reshape(4, 128).T)
    bab = np.ascontiguousarray(ba.reshape(2, 1), f32)
    return dict(wc=wc_host, wvu=wvu_host, wa=wa_host, bcb=bcb, bvub=bvub, bab=bab)


_NC_CACHE = {}


def get_nc(NL):
    if NL not in _NC_CACHE:
        _NC_CACHE[NL] = build_nc(NL)
    return _NC_CACHE[NL]


def run_cores(x, weights, n_cores=N_CORES):
    """x: [N, 1024] f32. Returns A [2, N], p [2, 512] (f64), Z [2] (f64)."""
    N = x.shape[0]
    NL = N // n_cores
    nc = get_nc(NL)
    wmap = prep_weights(**weights)
    xT = np.ascontiguousarray(x.T)
    in_maps = []
    for c in range(n_cores):
        m = dict(wmap)
        m["xt"] = np.ascontiguousarray(xT[:, c * NL:(c + 1) * NL])
        in_maps.append(m)
    res = run_bass_kernel_spmd(nc, in_maps, core_ids=list(range(n_cores)))
    A = np.concatenate([res.results[c]["a_out"] for c in range(n_cores)], axis=1)
    p_raw = np.sum([res.results[c]["p_out"].astype(np.float64) for c in range(n_cores)], axis=0)
    p_raw = p_raw.reshape(128, -1, 8).sum(axis=1)
    p = p_raw.reshape(128, 2, 4).transpose(1, 2, 0).reshape(2, 512)
    Z = np.sum([res.results[c]["s_out"].astype(np.float64).sum(axis=1) for c in range(n_cores)], axis=0)
    return A, p, Z


def host_finalize(A, p, Z, x, Wc, bc, Winst, binst, Wbag, bbag, label):
    bag = (p / Z[:, None]).astype(np.float32)                         # [2, 512]
    logits = (np.sum(bag * Wbag, axis=1) + bbag)[None, :].astype(np.float32)
    ex = np.exp((logits - logits.max()).astype(np.float32))
    y_proba = (ex / ex.sum()).astype(np.float32)

    attn_c = A[label]
    top_p = np.argsort(-attn_c, kind="stable")[:N_INST]
    top_n = np.argsort(attn_c, kind="stable")[:N_INST]
    idx = np.concatenate([top_p, top_n])
    h_rows = np.maximum(x[idx] @ Wc.T + bc, 0.0).astype(np.float32)
    logits_i = (h_rows @ Winst[label].T + binst[label]).astype(np.float32)
    targets = np.concatenate([np.ones(N_INST, np.int64), np.zeros(N_INST, np.int64)])
    onehot = np.eye(2, dtype=np.float32)[targets]
    z = (logits_i + ALPHA * (1.0 - onehot)) / TAU
    zm = z.max(axis=1, keepdims=True)
    lse = TAU * (np.log(np.exp(z - zm).sum(axis=1)) + zm[:, 0])
    s_y = logits_i[np.arange(2 * N_INST), targets]
    total_loss = np.float32((lse - s_y).mean())
    return logits, y_proba, total_loss


def _host_softmax_stats(A, x, Wc, bc):
    """Fallback: exact bag stats from A on the host (used only if the device's
    unshifted exp over/underflowed, i.e. |A| was far outside the expected
    O(1) range for this model). Recomputes h in row blocks."""
    m = A.max(axis=1, keepdims=True)
    e = np.exp((A - m).astype(np.float64))
    Z = e.sum(axis=1)
    p = np.zeros((2, Wc.shape[0]), np.float64)
    B = 8192
    for i in range(0, x.shape[0], B):
        h = np.maximum(x[i:i + B] @ Wc.T + bc, 0.0).astype(np.float32)
        p += e[:, i:i + B] @ h.astype(np.float64)
    return p, Z


def kernel(**inputs):
    x = np.ascontiguousarray(np.asarray(inputs["x"]), dtype=np.float32)
    label = int(inputs["label"])
    weights = {k: np.asarray(inputs[k], np.float32)
               for k in ["Wc", "bc", "Wv", "bv", "Wu", "bu", "Wa", "ba"]}
    A, p, Z = run_cores(x, weights)
    if not (np.all(np.isfinite(p)) and np.all(np.isfinite(Z)) and Z.min() > 0
            and np.abs(A).max() < 60.0):
        p, Z = _host_softmax_stats(A, x, weights["Wc"], weights["bc"])
    logits, y_proba, total_loss = host_finalize(
        A, p, Z, x, weights["Wc"], weights["bc"],
        np.asarray(inputs["Winst"], np.float32), np.asarray(inputs["binst"], np.float32),
        np.asarray(inputs["Wbag"], np.float32), np.asarray(inputs["bbag"], np.float32),
        label)
    return logits, y_proba, A, total_loss
